# revision 15
# baseline (speedup 1.0000x reference)
"""Trainium2 Bass kernel for nn_DeterministicDecoder.

Reference computation (B=8192, N=4096, D=2048, M=1024, C=64, K=100):
    k = emb @ W_k; v = emb @ W_v; q = feat @ W_q
    scores = (q @ k.T) / sqrt(M); a = softmax(scores, axis=1)
    attended = (a @ v) * 0.1 + q
    normed = LN(attended) * gamma + beta           (eps=1e-6)
    fc_hash = normed @ W_hash + b_hash
    prob = sigmoid(fc_hash); code = (prob > 0.5)   (straight-through => hard)
    fc_cls = code @ W_cls + b_cls
    returns (code, prob, fc_cls)

Strategy: data-parallel over B across 8 NeuronCores (1024 rows each);
emb and all weights replicated. Host pre-transposes feat/emb so the
contraction dim D lies on SBUF partitions. k/v/scores/attention run in
bf16 (error feeds through softmax attenuated by the 0.1 residual
scale); q runs as a 3-term bf16 hi/lo split (near-f32 accuracy at bf16
speed); layernorm, fc_hash, fc_cls run in f32 since fc_hash's sign
drives the binarized code output.

Perf notes (from NTFF traces):
 - one global PSUM pool (8 banks) avoids phase-boundary stalls
 - accumulation loops use 2 groups of 4 psum banks so evictions of one
   group overlap matmuls of the next
 - input streams ride nc.sync's HWDGE queue; bulk prefetch + output
   writes ride nc.gpsimd's queue so they never block the input stream
"""
import numpy as np
import ml_dtypes

import concourse.bass as bass
import concourse.bacc as bacc
import concourse.tile as tile
from concourse import mybir
from concourse.bass_utils import run_bass_kernel_spmd
from concourse.masks import make_identity

F32 = mybir.dt.float32
BF16 = mybir.dt.bfloat16
AF = mybir.ActivationFunctionType

P = 128
B_FULL, N, D, M, C, KC = 8192, 4096, 2048, 1024, 64, 100
NCORES = 8
B = B_FULL // NCORES           # 1024 rows per core
DC, BC, MC, NC_ = D // P, B // P, M // P, N // P   # 16, 8, 8, 32
BB = 256                       # batch block for the attention phase
NBB = B // BB                  # 4
BSUB = BB // P                 # 2
LN_EPS = 1e-6

_CACHED = {}


def _build():
    """Build + compile the per-core Bass program (identical on all cores)."""
    nc = bacc.Bacc("TRN2", target_bir_lowering=False, debug=False,
                   num_devices=NCORES)

    fth = nc.dram_tensor("fth", [D, B], BF16, kind="ExternalInput")
    ftl = nc.dram_tensor("ftl", [D, B], BF16, kind="ExternalInput")
    embT = nc.dram_tensor("embT", [D, N], BF16, kind="ExternalInput")
    Wk = nc.dram_tensor("Wk", [D, M], BF16, kind="ExternalInput")
    Wv = nc.dram_tensor("Wv", [D, M], BF16, kind="ExternalInput")
    Wqh = nc.dram_tensor("Wqh", [D, M], BF16, kind="ExternalInput")
    Wql = nc.dram_tensor("Wql", [D, M], BF16, kind="ExternalInput")
    Wh = nc.dram_tensor("Wh", [M, C], F32, kind="ExternalInput")
    bh = nc.dram_tensor("bh", [C, 1], F32, kind="ExternalInput")
    gamma = nc.dram_tensor("gamma", [1, M], F32, kind="ExternalInput")
    beta = nc.dram_tensor("beta", [1, M], F32, kind="ExternalInput")
    Wcls = nc.dram_tensor("Wcls", [P, KC], F32, kind="ExternalInput")  # zero-padded 64->128
    bcls = nc.dram_tensor("bcls", [1, KC], F32, kind="ExternalInput")

    code_o = nc.dram_tensor("code", [B, C], F32, kind="ExternalOutput")
    prob_o = nc.dram_tensor("prob", [B, C], F32, kind="ExternalOutput")
    fccls_o = nc.dram_tensor("fccls", [B, KC], F32, kind="ExternalOutput")

    with tile.TileContext(nc) as tc:
        _emit(nc, tc, fth, ftl, embT, Wk, Wv, Wqh, Wql, Wh, bh, gamma, beta,
              Wcls, bcls, code_o, prob_o, fccls_o)
    nc.compile()
    return nc


def _emit(nc, tc, fth, ftl, embT, Wk, Wv, Wqh, Wql, Wh, bh, gamma, beta,
          Wcls, bcls, code_o, prob_o, fccls_o):
    from contextlib import ExitStack

    with ExitStack() as ctx:
        const = ctx.enter_context(tc.tile_pool(name="const", bufs=1))
        ident = const.tile([P, P], F32)
        make_identity(nc, ident)
        ones_bf = const.tile([P, 1], BF16)
        nc.vector.memset(ones_bf, 1.0)
        gamma_t = const.tile([P, M], F32)
        nc.sync.dma_start(out=gamma_t, in_=gamma[:, :].to_broadcast([P, M]))
        beta_t = const.tile([P, M], F32)
        nc.sync.dma_start(out=beta_t, in_=beta[:, :].to_broadcast([P, M]))
        bh_t = const.tile([C, 1], F32)
        nc.sync.dma_start(out=bh_t, in_=bh[:, :])
        bcls_t = const.tile([P, KC], F32)
        nc.sync.dma_start(out=bcls_t, in_=bcls[:, :].to_broadcast([P, KC]))
        wh_t = const.tile([P, MC, C], F32)
        nc.sync.dma_start(out=wh_t, in_=Wh.rearrange("(mc p) c -> p mc c", p=P))
        wcls_t = const.tile([P, KC], F32)
        nc.sync.dma_start(out=wcls_t, in_=Wcls[:, :])
        eps_t = const.tile([P, 1], F32)
        nc.vector.memset(eps_t, LN_EPS)

        # DRAM bounce for q (f32) and qT (bf16)
        dram = ctx.enter_context(tc.tile_pool(name="dram", bufs=1, space="DRAM"))
        qd = dram.tile([B, M], F32)
        qTd = dram.tile([M, B], BF16)

        # One global PSUM pool for every phase: 8 banks of [P,512] f32.
        psG = ctx.enter_context(tc.tile_pool(name="psG", bufs=8, space="PSUM"))

        def psum(shape, name):
            return psG.tile(shape, F32, tag="ps", name=name)

        # Phase-1 weights prefetched on the gpsimd queue (doesn't block the
        # sync-queue input streams). Right SBUF stack: released after ph 1.
        pre = ExitStack()
        wkp = pre.enter_context(tc.tile_pool(name="wkp", bufs=1, side="right"))
        wvp = pre.enter_context(tc.tile_pool(name="wvp", bufs=1, side="right"))
        etp = pre.enter_context(tc.tile_pool(name="etp", bufs=3, side="right"))
        wk_t = wkp.tile([P, DC, M], BF16)       # 32KB/part
        nc.gpsimd.dma_start(out=wk_t, in_=Wk.rearrange("(dc p) m -> p dc m", p=P))
        wv_t = wvp.tile([P, DC, M], BF16)       # 32KB/part
        nc.gpsimd.dma_start(out=wv_t, in_=Wv.rearrange("(dc p) m -> p dc m", p=P))

        # ---------- Phase 0: q = feat @ W_q via bf16 hi/lo split ----------
        # q = fh@Wh + fl@Wh + fh@Wl  (bf16 products are exact in f32 psum;
        # dropped fl@Wl term ~2^-18 relative)
        with ExitStack() as p0:
            fp = p0.enter_context(tc.tile_pool(name="fp", bufs=1))
            fth_t = fp.tile([P, DC, B], BF16, name="fth")   # 32KB/part
            ftl_t = fp.tile([P, DC, B], BF16, name="ftl")   # 32KB/part
            for d in range(DC):
                nc.sync.dma_start(
                    out=fth_t[:, d, :],
                    in_=fth.rearrange("(dc p) b -> p dc b", p=P)[:, d, :])
                nc.sync.dma_start(
                    out=ftl_t[:, d, :],
                    in_=ftl.rearrange("(dc p) b -> p dc b", p=P)[:, d, :])
            wqp = p0.enter_context(tc.tile_pool(name="wqp", bufs=4))
            qsp = p0.enter_context(tc.tile_pool(name="qsp", bufs=1))
            q_sb = qsp.tile([P, BC, M], F32)    # 32KB/part
            qtp = p0.enter_context(tc.tile_pool(name="qtp", bufs=4))

            # 2 groups of 4 psum banks per mb-half so evictions overlap
            for mb in range(2):
                for bh4 in range(2):
                    ps = [psum([P, 512], f"q_ps{bh4}_{b}") for b in range(4)]
                    for d in range(DC):
                        wqh_t = wqp.tile([P, 512], BF16, tag="wq", name="wqh")
                        nc.sync.dma_start(
                            out=wqh_t,
                            in_=Wqh[d * P:(d + 1) * P, mb * 512:(mb + 1) * 512])
                        wql_t = wqp.tile([P, 512], BF16, tag="wq", name="wql")
                        nc.sync.dma_start(
                            out=wql_t,
                            in_=Wql[d * P:(d + 1) * P, mb * 512:(mb + 1) * 512])
                        for b4 in range(4):
                            b = bh4 * 4 + b4
                            fh = fth_t[:, d, b * P:(b + 1) * P]
                            fl = ftl_t[:, d, b * P:(b + 1) * P]
                            nc.tensor.matmul(ps[b4], fh, wqh_t,
                                             start=(d == 0), stop=False)
                            nc.tensor.matmul(ps[b4], fh, wql_t,
                                             start=False, stop=False)
                            nc.tensor.matmul(ps[b4], fl, wqh_t,
                                             start=False, stop=(d == DC - 1))
                    for b4 in range(4):
                        b = bh4 * 4 + b4
                        nc.vector.tensor_copy(
                            q_sb[:, b, mb * 512:(mb + 1) * 512], ps[b4])
            # q -> DRAM (gpsimd queue: write, not latency-critical)
            nc.gpsimd.dma_start(
                out=qd[:].rearrange("(bc p) m -> p bc m", p=P), in_=q_sb)
            # qT via PE transpose, cast bf16, -> DRAM
            for b in range(BC):
                for m in range(MC):
                    tps = psum([P, P], "q_tp")
                    nc.tensor.transpose(tps, q_sb[:, b, m * P:(m + 1) * P], ident)
                    qt_t = qtp.tile([P, P], BF16)
                    nc.vector.tensor_copy(qt_t, tps)
                    nc.gpsimd.dma_start(
                        out=qTd[m * P:(m + 1) * P, b * P:(b + 1) * P], in_=qt_t)

        # Big SBUF residents for the attention phase (alloc after phase 0
        # pools release -- stack discipline)
        big = ctx.enter_context(tc.tile_pool(name="big", bufs=1))
        kT_sb = big.tile([P, MC, N], BF16)      # kT[m, n] : 64KB/part
        v_sb = big.tile([P, NC_, M], BF16)      # v[n, m]  : 64KB/part

        # ---------- Phase 1: kT = W_k.T @ embT ; v = embT.T @ W_v ----------
        # m/ns-halves of 4 psum banks; embT tiles re-streamed per half so
        # evictions of one half overlap matmuls of the next.
        for nb in range(N // 512):
            for mh in range(2):
                ps = [psum([P, 512], f"k_ps{mh}_{m}") for m in range(4)]
                for d in range(DC):
                    et = etp.tile([P, 512], BF16)
                    nc.sync.dma_start(
                        out=et,
                        in_=embT[d * P:(d + 1) * P, nb * 512:(nb + 1) * 512])
                    for m4 in range(4):
                        m = mh * 4 + m4
                        nc.tensor.matmul(ps[m4], wk_t[:, d, m * P:(m + 1) * P],
                                         et, start=(d == 0), stop=(d == DC - 1))
                for m4 in range(4):
                    m = mh * 4 + m4
                    nc.vector.tensor_copy(
                        kT_sb[:, m, nb * 512:(nb + 1) * 512], ps[m4])

        for nb in range(N // 512):
            for g in range(2):  # 2 n-subchunks x 2 m-halves per group
                ps = [psum([P, 512], f"v_ps{g}_{i}") for i in range(4)]
                for d in range(DC):
                    et = etp.tile([P, 512], BF16)
                    nc.sync.dma_start(
                        out=et,
                        in_=embT[d * P:(d + 1) * P, nb * 512:(nb + 1) * 512])
                    for i in range(4):
                        ns = g * 2 + i // 2
                        mh = i % 2
                        nc.tensor.matmul(
                            ps[i], et[:, ns * P:(ns + 1) * P],
                            wv_t[:, d, mh * 512:(mh + 1) * 512],
                            start=(d == 0), stop=(d == DC - 1))
                for i in range(4):
                    ns = g * 2 + i // 2
                    mh = i % 2
                    nc.vector.tensor_copy(
                        v_sb[:, nb * 4 + ns, mh * 512:(mh + 1) * 512], ps[i])

        # Release the right-stack prefetch pools before phase 2 allocates.
        pre.close()

        # ---------- Phase 2: attention + LN + heads, per batch block ----------
        with ExitStack() as p2:
            # expT in 2 half-tiles (16 n-chunks each) on the right stack
            expp = p2.enter_context(tc.tile_pool(name="expp", bufs=3,
                                                 side="right"))
            qtp2 = p2.enter_context(tc.tile_pool(name="qtp2", bufs=2))
            qp2 = p2.enter_context(tc.tile_pool(name="qp2", bufs=1))
            atp = p2.enter_context(tc.tile_pool(name="atp", bufs=1))
            ntp = p2.enter_context(tc.tile_pool(name="ntp", bufs=1))
            smal = p2.enter_context(tc.tile_pool(name="smal", bufs=2))
            outp = p2.enter_context(tc.tile_pool(name="outp", bufs=2))

            for bb in range(NBB):
                # scoresT[n, b] = kT.T @ qT ; exp via ACT (scale=1/sqrt(M))
                qt_t = qtp2.tile([P, MC, BB], BF16)
                nc.sync.dma_start(
                    out=qt_t,
                    in_=qTd[:, bb * BB:(bb + 1) * BB].rearrange(
                        "(mc p) b -> p mc b", p=P))
                exp_h = [expp.tile([P, NC_ // 2, BB], BF16, tag="exp",
                                   name=f"exp{h}") for h in range(2)]
                for c in range(NC_):
                    pss = psum([P, BB], "s_ps")
                    for m in range(MC):
                        nc.tensor.matmul(pss, kT_sb[:, m, c * P:(c + 1) * P],
                                         qt_t[:, m, :], start=(m == 0),
                                         stop=(m == MC - 1))
                    nc.scalar.activation(exp_h[c // 16][:, c % 16, :], pss,
                                         AF.Exp, scale=1.0 / float(np.sqrt(M)))

                # attended = (expT.T @ v) / den * 0.1 + q ; then LN
                q_t = qp2.tile([P, BSUB, M], F32)
                nc.sync.dma_start(
                    out=q_t,
                    in_=qd[bb * BB:(bb + 1) * BB, :].rearrange(
                        "(bs p) m -> p bs m", p=P))
                at_t = atp.tile([P, BSUB, M], F32)
                nt_t = ntp.tile([P, MC, BB], F32)
                for bs in range(BSUB):
                    psd = psum([P, 1], "d_ps")
                    psm = [psum([P, 512], f"at_ps{mh}") for mh in range(2)]
                    for c in range(NC_):
                        lhs = exp_h[c // 16][:, c % 16, bs * P:(bs + 1) * P]
                        nc.tensor.matmul(psm[0], lhs, v_sb[:, c, 0:512],
                                         start=(c == 0), stop=(c == NC_ - 1))
                        nc.tensor.matmul(psm[1], lhs, v_sb[:, c, 512:1024],
                                         start=(c == 0), stop=(c == NC_ - 1))
                        nc.tensor.matmul(psd, lhs, ones_bf,
                                         start=(c == 0), stop=(c == NC_ - 1))
                    # recip(den) * 0.1
                    rec = smal.tile([P, 1], F32, tag="sm1")
                    nc.vector.reciprocal(out=rec, in_=psd)
                    rec01 = smal.tile([P, 1], F32, tag="sm1")
                    nc.scalar.mul(rec01, rec, 0.1)
                    for mh in range(2):
                        sl = slice(mh * 512, (mh + 1) * 512)
                        nc.vector.tensor_scalar_mul(
                            out=at_t[:, bs, sl], in0=psm[mh], scalar1=rec01)
                    nc.vector.tensor_add(out=at_t[:, bs, :],
                                         in0=at_t[:, bs, :], in1=q_t[:, bs, :])
                    # LayerNorm over M
                    stats = smal.tile([P, 2, 6], F32, tag="sm6")
                    for h in range(2):
                        nc.vector.bn_stats(out=stats[:, h, :],
                                           in_=at_t[:, bs, h * 512:(h + 1) * 512])
                    mv = smal.tile([P, 2], F32, tag="sm2")
                    nc.vector.bn_aggr(out=mv, in_=stats)
                    rstd = smal.tile([P, 1], F32, tag="sm1")
                    nc.scalar.activation(rstd, mv[:, 1:2], AF.Sqrt, bias=eps_t)
                    nc.vector.reciprocal(out=rstd, in_=rstd)
                    nc.vector.tensor_scalar(
                        out=at_t[:, bs, :], in0=at_t[:, bs, :],
                        scalar1=mv[:, 0:1], scalar2=rstd,
                        op0=mybir.AluOpType.subtract, op1=mybir.AluOpType.mult)
                    nc.vector.tensor_mul(out=at_t[:, bs, :],
                                         in0=at_t[:, bs, :], in1=gamma_t)
                    nc.vector.tensor_add(out=at_t[:, bs, :],
                                         in0=at_t[:, bs, :], in1=beta_t)
                    # transpose normed -> nt_t[m, bs*P:...]
                    for m in range(MC):
                        tp = psum([P, P], "n_tp")
                        nc.tensor.transpose(tp, at_t[:, bs, m * P:(m + 1) * P],
                                            ident)
                        nc.vector.tensor_copy(
                            nt_t[:, m, bs * P:(bs + 1) * P], tp)

                # fc_hashT[c_hash, b] (f32) + sigmoid / sign + fc_cls
                psh = psum([P, BB], "h_ps")
                for m in range(MC):
                    nc.tensor.matmul(psh[:C, :], wh_t[:, m, :], nt_t[:, m, :],
                                     start=(m == 0), stop=(m == MC - 1))
                probT = smal.tile([C, BB], F32, tag="smp", name="probT")
                nc.scalar.activation(probT, psh[:C, :], AF.Sigmoid, bias=bh_t)
                hashT = smal.tile([C, BB], F32, tag="smp", name="hashT")
                nc.vector.tensor_scalar(out=hashT, in0=psh[:C, :],
                                        scalar1=bh_t, scalar2=None,
                                        op0=mybir.AluOpType.add)
                codeT = smal.tile([P, BB], F32, tag="smc", name="codeT")
                nc.vector.memset(codeT[C:, :], 0.0)
                nc.vector.tensor_scalar(out=codeT[:C, :], in0=hashT,
                                        scalar1=0.0, scalar2=None,
                                        op0=mybir.AluOpType.is_gt)

                for bs in range(BSUB):
                    # fc_cls = codeT.T @ Wcls + bcls
                    psc = psum([P, P], "c_ps")
                    nc.tensor.matmul(psc[:, :KC],
                                     codeT[:, bs * P:(bs + 1) * P], wcls_t,
                                     start=True, stop=True)
                    fc_t = outp.tile([P, KC], F32, tag="ofc")
                    nc.vector.tensor_add(out=fc_t, in0=psc[:, :KC], in1=bcls_t)
                    nc.gpsimd.dma_start(
                        out=fccls_o[bb * BB + bs * P: bb * BB + (bs + 1) * P, :],
                        in_=fc_t)
                    # transpose prob/code back to [b, C]
                    for src, dst in ((probT, prob_o), (codeT, code_o)):
                        tp = psum([P, P], "o_tp")
                        nc.tensor.transpose(tp[:, :C],
                                            src[:C, bs * P:(bs + 1) * P],
                                            ident[:C, :C])
                        ot = outp.tile([P, C], F32, tag="oc")
                        nc.vector.tensor_copy(ot, tp[:, :C])
                        nc.gpsimd.dma_start(
                            out=dst[bb * BB + bs * P: bb * BB + (bs + 1) * P, :],
                            in_=ot)


def _prep_inputs(feat, emb, W_k, W_v, W_q, W_hash, b_hash, ln_gamma, ln_beta,
                 W_cls, b_cls):
    bf = ml_dtypes.bfloat16
    embT = np.ascontiguousarray(emb.T).astype(bf)
    Wk_b = np.ascontiguousarray(W_k).astype(bf)
    Wv_b = np.ascontiguousarray(W_v).astype(bf)
    Wq_f = np.ascontiguousarray(W_q, dtype=np.float32)
    Wqh_b = Wq_f.astype(bf)
    Wql_b = (Wq_f - Wqh_b.astype(np.float32)).astype(bf)
    Wh_f = np.ascontiguousarray(W_hash, dtype=np.float32)
    bh_f = np.ascontiguousarray(b_hash, dtype=np.float32).reshape(C, 1)
    gamma_f = np.ascontiguousarray(ln_gamma, dtype=np.float32).reshape(1, M)
    beta_f = np.ascontiguousarray(ln_beta, dtype=np.float32).reshape(1, M)
    Wcls_pad = np.zeros((P, KC), dtype=np.float32)
    Wcls_pad[:C, :] = W_cls
    bcls_f = np.ascontiguousarray(b_cls, dtype=np.float32).reshape(1, KC)

    in_maps = []
    for i in range(NCORES):
        featT_i = np.ascontiguousarray(
            feat[i * B:(i + 1) * B, :].T).astype(np.float32)
        fth_i = featT_i.astype(bf)
        ftl_i = (featT_i - fth_i.astype(np.float32)).astype(bf)
        in_maps.append({
            "fth": fth_i, "ftl": ftl_i, "embT": embT, "Wk": Wk_b, "Wv": Wv_b,
            "Wqh": Wqh_b, "Wql": Wql_b, "Wh": Wh_f, "bh": bh_f,
            "gamma": gamma_f, "beta": beta_f, "Wcls": Wcls_pad, "bcls": bcls_f,
        })
    return in_maps


def kernel(feat, emb, W_k, W_v, W_q, W_hash, b_hash, ln_gamma, ln_beta,
           W_cls, b_cls):
    feat = np.asarray(feat, dtype=np.float32)
    emb = np.asarray(emb, dtype=np.float32)
    in_maps = _prep_inputs(feat, emb, np.asarray(W_k), np.asarray(W_v),
                           np.asarray(W_q), np.asarray(W_hash),
                           np.asarray(b_hash), np.asarray(ln_gamma),
                           np.asarray(ln_beta), np.asarray(W_cls),
                           np.asarray(b_cls))
    if "nc" not in _CACHED:
        _CACHED["nc"] = _build()
    res = run_bass_kernel_spmd(_CACHED["nc"], in_maps, list(range(NCORES)))
    code = np.concatenate([r["code"] for r in res.results], axis=0)
    prob = np.concatenate([r["prob"] for r in res.results], axis=0)
    fccls = np.concatenate([r["fccls"] for r in res.results], axis=0)
    return code, prob, fccls


# Exposed for test.py profiling runs
def run_profiled(feat, emb, W_k, W_v, W_q, W_hash, b_hash, ln_gamma, ln_beta,
                 W_cls, b_cls, trace=True):
    in_maps = _prep_inputs(np.asarray(feat, dtype=np.float32),
                           np.asarray(emb, dtype=np.float32),
                           np.asarray(W_k), np.asarray(W_v), np.asarray(W_q),
                           np.asarray(W_hash), np.asarray(b_hash),
                           np.asarray(ln_gamma), np.asarray(ln_beta),
                           np.asarray(W_cls), np.asarray(b_cls))
    if "nc" not in _CACHED:
        _CACHED["nc"] = _build()
    res = run_bass_kernel_spmd(_CACHED["nc"], in_maps, list(range(NCORES)),
                               trace=trace)
    code = np.concatenate([r["code"] for r in res.results], axis=0)
    prob = np.concatenate([r["prob"] for r in res.results], axis=0)
    fccls = np.concatenate([r["fccls"] for r in res.results], axis=0)
    return (code, prob, fccls), res


# revision 23
# speedup vs baseline: 1.9155x; 1.9155x over previous
"""Trainium2 Bass kernel for nn_DeterministicDecoder.

Reference computation (B=8192, N=4096, D=2048, M=1024, C=64, K=100):
    k = emb @ W_k; v = emb @ W_v; q = feat @ W_q
    scores = (q @ k.T) / sqrt(M); a = softmax(scores, axis=1)
    attended = (a @ v) * 0.1 + q
    normed = LN(attended) * gamma + beta           (eps=1e-6)
    fc_hash = normed @ W_hash + b_hash
    prob = sigmoid(fc_hash); code = (prob > 0.5)   (straight-through => hard)
    fc_cls = code @ W_cls + b_cls
    returns (code, prob, fc_cls)

Strategy: data-parallel over B across 8 NeuronCores (1024 rows each);
emb and all weights replicated. Host pre-transposes feat/emb so the
contraction dim D lies on SBUF partitions. k/v/scores/attention run in
bf16 (error feeds through softmax attenuated by the 0.1 residual
scale); q, layernorm, fc_hash, fc_cls run in f32 since fc_hash's sign
drives the binarized code output.
"""
import numpy as np
import ml_dtypes

import concourse.bass as bass
import concourse.bacc as bacc
import concourse.tile as tile
from concourse import mybir
from concourse.bass_utils import run_bass_kernel_spmd
from concourse.masks import make_identity

F32 = mybir.dt.float32
BF16 = mybir.dt.bfloat16
AF = mybir.ActivationFunctionType

P = 128
B_FULL, N, D, M, C, KC = 8192, 4096, 2048, 1024, 64, 100
NCORES = 8
B = B_FULL // NCORES           # 1024 rows per core
DC, BC, MC, NC_ = D // P, B // P, M // P, N // P   # 16, 8, 8, 32
BB = 256                       # batch block for the attention phase
NBB = B // BB                  # 4
BSUB = BB // P                 # 2
LN_EPS = 1e-6

_CACHED = {}


def _build():
    """Build + compile the per-core Bass program (identical on all cores)."""
    nc = bacc.Bacc("TRN2", target_bir_lowering=False, debug=False,
                   num_devices=NCORES)

    fth = nc.dram_tensor("fth", [D, B], BF16, kind="ExternalInput")
    ftl = nc.dram_tensor("ftl", [D, B], BF16, kind="ExternalInput")
    embT = nc.dram_tensor("embT", [D, N], BF16, kind="ExternalInput")
    Wk = nc.dram_tensor("Wk", [D, M], BF16, kind="ExternalInput")
    Wv = nc.dram_tensor("Wv", [D, M], BF16, kind="ExternalInput")
    Wqh = nc.dram_tensor("Wqh", [D, M], BF16, kind="ExternalInput")
    Wql = nc.dram_tensor("Wql", [D, M], BF16, kind="ExternalInput")
    Wh = nc.dram_tensor("Wh", [M, C], F32, kind="ExternalInput")
    bh = nc.dram_tensor("bh", [C, 1], F32, kind="ExternalInput")
    gamma = nc.dram_tensor("gamma", [1, M], F32, kind="ExternalInput")
    beta = nc.dram_tensor("beta", [1, M], F32, kind="ExternalInput")
    Wcls = nc.dram_tensor("Wcls", [P, KC], F32, kind="ExternalInput")  # zero-padded 64->128
    bcls = nc.dram_tensor("bcls", [1, KC], F32, kind="ExternalInput")

    code_o = nc.dram_tensor("code", [B, C], F32, kind="ExternalOutput")
    prob_o = nc.dram_tensor("prob", [B, C], F32, kind="ExternalOutput")
    fccls_o = nc.dram_tensor("fccls", [B, KC], F32, kind="ExternalOutput")

    with tile.TileContext(nc) as tc:
        _emit(nc, tc, fth, ftl, embT, Wk, Wv, Wqh, Wql, Wh, bh, gamma, beta,
              Wcls, bcls, code_o, prob_o, fccls_o)
    nc.compile()
    return nc


def _emit(nc, tc, fth, ftl, embT, Wk, Wv, Wqh, Wql, Wh, bh, gamma, beta,
          Wcls, bcls, code_o, prob_o, fccls_o):
    from contextlib import ExitStack

    with ExitStack() as ctx:
        const = ctx.enter_context(tc.tile_pool(name="const", bufs=1))
        ident = const.tile([P, P], F32)
        make_identity(nc, ident)
        ones_bf = const.tile([P, 1], BF16)
        nc.vector.memset(ones_bf, 1.0)
        gamma_t = const.tile([P, M], F32)
        nc.sync.dma_start(out=gamma_t, in_=gamma[:, :].to_broadcast([P, M]))
        beta_t = const.tile([P, M], F32)
        nc.sync.dma_start(out=beta_t, in_=beta[:, :].to_broadcast([P, M]))
        bh_t = const.tile([C, 1], F32)
        nc.sync.dma_start(out=bh_t, in_=bh[:, :])
        bcls_t = const.tile([P, KC], F32)
        nc.sync.dma_start(out=bcls_t, in_=bcls[:, :].to_broadcast([P, KC]))
        wh_t = const.tile([P, MC, C], F32)
        nc.sync.dma_start(out=wh_t, in_=Wh.rearrange("(mc p) c -> p mc c", p=P))
        wcls_t = const.tile([P, KC], F32)
        nc.sync.dma_start(out=wcls_t, in_=Wcls[:, :])
        eps_t = const.tile([P, 1], F32)
        nc.vector.memset(eps_t, LN_EPS)

        # DRAM bounce for q (f32) and qT (bf16)
        dram = ctx.enter_context(tc.tile_pool(name="dram", bufs=1, space="DRAM"))
        qd = dram.tile([B, M], F32)
        qTd = dram.tile([M, B], BF16)

        # Prefetch pools for phase 1 weights (allocated before phase 0 so
        # their DMAs overlap phase-0 compute; released after phase 1).
        # They live on the RIGHT SBUF stack so their release order is
        # independent of the left-stack phase pools.
        pre = ExitStack()
        wkp = pre.enter_context(tc.tile_pool(name="wkp", bufs=1, side="right"))
        wvp = pre.enter_context(tc.tile_pool(name="wvp", bufs=1, side="right"))
        etp = pre.enter_context(tc.tile_pool(name="etp", bufs=3, side="right"))
        # gpsimd queue: stays off the latency-critical sync input stream
        wk_t = wkp.tile([P, DC, M], BF16)       # 32KB/part
        nc.gpsimd.dma_start(out=wk_t, in_=Wk.rearrange("(dc p) m -> p dc m", p=P))
        wv_t = wvp.tile([P, DC, M], BF16)       # 32KB/part
        nc.gpsimd.dma_start(out=wv_t, in_=Wv.rearrange("(dc p) m -> p dc m", p=P))

        # ---------- Phase 0: q = feat @ W_q via bf16 hi/lo split ----------
        # q = fh@Wh + fl@Wh + fh@Wl  (bf16 products are exact in f32 psum;
        # dropped fl@Wl term ~2^-18 relative)
        with ExitStack() as p0:
            fp = p0.enter_context(tc.tile_pool(name="fp", bufs=1))
            fth_t = fp.tile([P, DC, B], BF16, name="fth")   # 32KB/part
            ftl_t = fp.tile([P, DC, B], BF16, name="ftl")   # 32KB/part
            # wqp holds a full mb-half of W_q hi+lo (32 tiles) + lookahead
            wqp = p0.enter_context(tc.tile_pool(name="wqp", bufs=12))
            qsp = p0.enter_context(tc.tile_pool(name="qsp", bufs=1))
            q_sb = qsp.tile([P, BC, M], F32)    # 32KB/part
            qtp = p0.enter_context(tc.tile_pool(name="qtp", bufs=4))
            ps0 = p0.enter_context(tc.tile_pool(name="ps0", bufs=8, space="PSUM"))

            def dma_wq(mb):
                tiles = []
                for d in range(DC):
                    wqh_t = wqp.tile([P, 512], BF16, tag="wq", name="wqh")
                    nc.sync.dma_start(
                        out=wqh_t,
                        in_=Wqh[d * P:(d + 1) * P, mb * 512:(mb + 1) * 512])
                    wql_t = wqp.tile([P, 512], BF16, tag="wq", name="wql")
                    nc.sync.dma_start(
                        out=wql_t,
                        in_=Wql[d * P:(d + 1) * P, mb * 512:(mb + 1) * 512])
                    tiles.append((wqh_t, wql_t))
                return tiles

            # Interleave the input DMAs so the d=0 operands land first and
            # the first matmul starts ~2us in.
            wq_tiles = {0: []}
            for d in range(DC):
                nc.sync.dma_start(
                    out=fth_t[:, d, :],
                    in_=fth.rearrange("(dc p) b -> p dc b", p=P)[:, d, :])
                nc.sync.dma_start(
                    out=ftl_t[:, d, :],
                    in_=ftl.rearrange("(dc p) b -> p dc b", p=P)[:, d, :])
                wqh_t = wqp.tile([P, 512], BF16, tag="wq", name="wqh")
                nc.sync.dma_start(out=wqh_t,
                                  in_=Wqh[d * P:(d + 1) * P, 0:512])
                wql_t = wqp.tile([P, 512], BF16, tag="wq", name="wql")
                nc.sync.dma_start(out=wql_t,
                                  in_=Wql[d * P:(d + 1) * P, 0:512])
                wq_tiles[0].append((wqh_t, wql_t))

            for mb in range(2):
                if mb not in wq_tiles:
                    wq_tiles[mb] = dma_wq(mb)
                ps = [ps0.tile([P, 512], F32, tag="ps0", name=f"q_ps{b}")
                      for b in range(BC)]
                for d in range(DC):
                    wqh_t, wql_t = wq_tiles[mb][d]
                    for b in range(BC):
                        fh = fth_t[:, d, b * P:(b + 1) * P]
                        fl = ftl_t[:, d, b * P:(b + 1) * P]
                        nc.tensor.matmul(ps[b], fh, wqh_t, start=(d == 0),
                                         stop=False)
                        nc.tensor.matmul(ps[b], fh, wql_t, start=False,
                                         stop=False)
                        nc.tensor.matmul(ps[b], fl, wqh_t, start=False,
                                         stop=(d == DC - 1))
                for b in range(BC):
                    nc.vector.tensor_copy(q_sb[:, b, mb * 512:(mb + 1) * 512],
                                          ps[b])
            # q -> DRAM (gpsimd queue: writes stay off the input stream)
            nc.gpsimd.dma_start(
                out=qd[:].rearrange("(bc p) m -> p bc m", p=P), in_=q_sb)
            # qT via PE transpose, cast bf16, -> DRAM
            for b in range(BC):
                for m in range(MC):
                    tps = ps0.tile([P, P], F32, tag="ps0", name="q_tp")
                    nc.tensor.transpose(tps, q_sb[:, b, m * P:(m + 1) * P], ident)
                    qt_t = qtp.tile([P, P], BF16)
                    nc.vector.tensor_copy(qt_t, tps)
                    nc.gpsimd.dma_start(
                        out=qTd[m * P:(m + 1) * P, b * P:(b + 1) * P], in_=qt_t)

        # Big SBUF residents for the attention phase (alloc after phase 0
        # pools release -- stack discipline)
        big = ctx.enter_context(tc.tile_pool(name="big", bufs=1))
        kT_sb = big.tile([P, MC, N], BF16)      # kT[m, n] : 64KB/part
        v_sb = big.tile([P, NC_, M], BF16)      # v[n, m]  : 64KB/part

        # ---------- Phase 1: kT = W_k.T @ embT ; v = embT.T @ W_v ----------
        with ExitStack() as p1:
            ps1 = p1.enter_context(tc.tile_pool(name="ps1", bufs=8, space="PSUM"))

            for nb in range(N // 512):
                ps = [ps1.tile([P, 512], F32, tag="ps1", name=f"k_ps{m}")
                      for m in range(MC)]
                for d in range(DC):
                    et = etp.tile([P, 512], BF16)
                    nc.sync.dma_start(
                        out=et,
                        in_=embT[d * P:(d + 1) * P, nb * 512:(nb + 1) * 512])
                    for m in range(MC):
                        nc.tensor.matmul(ps[m], wk_t[:, d, m * P:(m + 1) * P],
                                         et, start=(d == 0), stop=(d == DC - 1))
                for m in range(MC):
                    nc.vector.tensor_copy(kT_sb[:, m, nb * 512:(nb + 1) * 512],
                                          ps[m])

            for nb in range(N // 512):
                ps = [ps1.tile([P, 512], F32, tag="ps1", name=f"v_ps{i}")
                      for i in range(8)]  # 4 n-subchunks x 2 m-halves
                for d in range(DC):
                    et = etp.tile([P, 512], BF16)
                    nc.sync.dma_start(
                        out=et,
                        in_=embT[d * P:(d + 1) * P, nb * 512:(nb + 1) * 512])
                    for ns in range(4):
                        for mh in range(2):
                            nc.tensor.matmul(
                                ps[ns * 2 + mh], et[:, ns * P:(ns + 1) * P],
                                wv_t[:, d, mh * 512:(mh + 1) * 512],
                                start=(d == 0), stop=(d == DC - 1))
                for ns in range(4):
                    for mh in range(2):
                        nc.vector.tensor_copy(
                            v_sb[:, nb * 4 + ns, mh * 512:(mh + 1) * 512],
                            ps[ns * 2 + mh])

        # Release the right-stack prefetch pools before phase 2 allocates.
        pre.close()

        # ---------- Phase 2: attention + LN + heads, per batch block ----------
        with ExitStack() as p2:
            ps_s = p2.enter_context(tc.tile_pool(name="ps_s", bufs=2, space="PSUM"))
            ps_m = p2.enter_context(tc.tile_pool(name="ps_m", bufs=2, space="PSUM"))
            ps_d = p2.enter_context(tc.tile_pool(name="ps_d", bufs=1, space="PSUM"))
            ps_x = p2.enter_context(tc.tile_pool(name="ps_x", bufs=2, space="PSUM"))
            expp = p2.enter_context(tc.tile_pool(name="expp", bufs=3,
                                                 side="right"))
            qtp2 = p2.enter_context(tc.tile_pool(name="qtp2", bufs=2))
            qp2 = p2.enter_context(tc.tile_pool(name="qp2", bufs=1))
            atp = p2.enter_context(tc.tile_pool(name="atp", bufs=1))
            ntp = p2.enter_context(tc.tile_pool(name="ntp", bufs=1))
            smal = p2.enter_context(tc.tile_pool(name="smal", bufs=2))
            outp = p2.enter_context(tc.tile_pool(name="outp", bufs=2))

            for bb in range(NBB):
                # scoresT[n, b] = kT.T @ qT ; exp via ACT (scale=1/sqrt(M))
                qt_t = qtp2.tile([P, MC, BB], BF16)
                nc.sync.dma_start(
                    out=qt_t,
                    in_=qTd[:, bb * BB:(bb + 1) * BB].rearrange(
                        "(mc p) b -> p mc b", p=P))
                exp_h = [expp.tile([P, NC_ // 2, BB], BF16, tag="exp",
                                   name=f"exp{h}") for h in range(2)]
                for c in range(NC_):
                    pss = ps_s.tile([P, BB], F32)
                    for m in range(MC):
                        nc.tensor.matmul(pss, kT_sb[:, m, c * P:(c + 1) * P],
                                         qt_t[:, m, :], start=(m == 0),
                                         stop=(m == MC - 1))
                    nc.scalar.activation(exp_h[c // 16][:, c % 16, :], pss,
                                         AF.Exp, scale=1.0 / float(np.sqrt(M)))

                # attended = (expT.T @ v) / den * 0.1 + q ; then LN
                q_t = qp2.tile([P, BSUB, M], F32)
                nc.sync.dma_start(
                    out=q_t,
                    in_=qd[bb * BB:(bb + 1) * BB, :].rearrange(
                        "(bs p) m -> p bs m", p=P))
                at_t = atp.tile([P, BSUB, M], F32)
                nt_t = ntp.tile([P, MC, BB], F32)
                for bs in range(BSUB):
                    psd = ps_d.tile([P, 1], F32)
                    psm = [ps_m.tile([P, 512], F32, tag="psm", name=f"at_ps{mh}")
                           for mh in range(2)]
                    for c in range(NC_):
                        lhs = exp_h[c // 16][:, c % 16, bs * P:(bs + 1) * P]
                        nc.tensor.matmul(psm[0], lhs, v_sb[:, c, 0:512],
                                         start=(c == 0), stop=(c == NC_ - 1))
                        nc.tensor.matmul(psm[1], lhs, v_sb[:, c, 512:1024],
                                         start=(c == 0), stop=(c == NC_ - 1))
                        nc.tensor.matmul(psd, lhs, ones_bf,
                                         start=(c == 0), stop=(c == NC_ - 1))
                    # recip(den) * 0.1
                    rec = smal.tile([P, 1], F32, tag="sm1")
                    nc.vector.reciprocal(out=rec, in_=psd)
                    rec01 = smal.tile([P, 1], F32, tag="sm1")
                    nc.scalar.mul(rec01, rec, 0.1)
                    for mh in range(2):
                        sl = slice(mh * 512, (mh + 1) * 512)
                        nc.vector.tensor_scalar_mul(
                            out=at_t[:, bs, sl], in0=psm[mh], scalar1=rec01)
                    nc.vector.tensor_add(out=at_t[:, bs, :],
                                         in0=at_t[:, bs, :], in1=q_t[:, bs, :])
                    # LayerNorm over M
                    stats = smal.tile([P, 2, 6], F32, tag="sm6")
                    for h in range(2):
                        nc.vector.bn_stats(out=stats[:, h, :],
                                           in_=at_t[:, bs, h * 512:(h + 1) * 512])
                    mv = smal.tile([P, 2], F32, tag="sm2")
                    nc.vector.bn_aggr(out=mv, in_=stats)
                    rstd = smal.tile([P, 1], F32, tag="sm1")
                    nc.scalar.activation(rstd, mv[:, 1:2], AF.Sqrt, bias=eps_t)
                    nc.vector.reciprocal(out=rstd, in_=rstd)
                    nc.vector.tensor_scalar(
                        out=at_t[:, bs, :], in0=at_t[:, bs, :],
                        scalar1=mv[:, 0:1], scalar2=rstd,
                        op0=mybir.AluOpType.subtract, op1=mybir.AluOpType.mult)
                    nc.vector.tensor_mul(out=at_t[:, bs, :],
                                         in0=at_t[:, bs, :], in1=gamma_t)
                    nc.vector.tensor_add(out=at_t[:, bs, :],
                                         in0=at_t[:, bs, :], in1=beta_t)
                    # transpose normed -> nt_t[m, bs*P:...]
                    for m in range(MC):
                        tp = ps_x.tile([P, P], F32, tag="psx")
                        nc.tensor.transpose(tp, at_t[:, bs, m * P:(m + 1) * P],
                                            ident)
                        nc.vector.tensor_copy(
                            nt_t[:, m, bs * P:(bs + 1) * P], tp)

                # fc_hashT[c_hash, b] (f32) + sigmoid / sign + fc_cls
                psh = ps_x.tile([P, BB], F32, tag="psx", name="hash_ps")
                for m in range(MC):
                    nc.tensor.matmul(psh[:C, :], wh_t[:, m, :], nt_t[:, m, :],
                                     start=(m == 0), stop=(m == MC - 1))
                probT = smal.tile([C, BB], F32, tag="smp", name="probT")
                nc.scalar.activation(probT, psh[:C, :], AF.Sigmoid, bias=bh_t)
                hashT = smal.tile([C, BB], F32, tag="smp", name="hashT")
                nc.vector.tensor_scalar(out=hashT, in0=psh[:C, :],
                                        scalar1=bh_t, scalar2=None,
                                        op0=mybir.AluOpType.add)
                codeT = smal.tile([P, BB], F32, tag="smc", name="codeT")
                nc.vector.memset(codeT[C:, :], 0.0)
                nc.vector.tensor_scalar(out=codeT[:C, :], in0=hashT,
                                        scalar1=0.0, scalar2=None,
                                        op0=mybir.AluOpType.is_gt)

                for bs in range(BSUB):
                    # fc_cls = codeT.T @ Wcls + bcls
                    psc = ps_x.tile([P, P], F32, tag="psx", name="cls_ps")
                    nc.tensor.matmul(psc[:, :KC],
                                     codeT[:, bs * P:(bs + 1) * P], wcls_t,
                                     start=True, stop=True)
                    fc_t = outp.tile([P, KC], F32, tag="ofc")
                    nc.vector.tensor_add(out=fc_t, in0=psc[:, :KC], in1=bcls_t)
                    nc.gpsimd.dma_start(
                        out=fccls_o[bb * BB + bs * P: bb * BB + (bs + 1) * P, :],
                        in_=fc_t)
                    # transpose prob/code back to [b, C]
                    for src, dst in ((probT, prob_o), (codeT, code_o)):
                        tp = ps_x.tile([P, P], F32, tag="psx", name="out_tp")
                        nc.tensor.transpose(tp[:, :C],
                                            src[:C, bs * P:(bs + 1) * P],
                                            ident[:C, :C])
                        ot = outp.tile([P, C], F32, tag="oc")
                        nc.vector.tensor_copy(ot, tp[:, :C])
                        nc.gpsimd.dma_start(
                            out=dst[bb * BB + bs * P: bb * BB + (bs + 1) * P, :],
                            in_=ot)


def _prep_inputs(feat, emb, W_k, W_v, W_q, W_hash, b_hash, ln_gamma, ln_beta,
                 W_cls, b_cls):
    bf = ml_dtypes.bfloat16
    embT = np.ascontiguousarray(emb.T).astype(bf)
    Wk_b = np.ascontiguousarray(W_k).astype(bf)
    Wv_b = np.ascontiguousarray(W_v).astype(bf)
    Wq_f = np.ascontiguousarray(W_q, dtype=np.float32)
    Wqh_b = Wq_f.astype(bf)
    Wql_b = (Wq_f - Wqh_b.astype(np.float32)).astype(bf)
    Wh_f = np.ascontiguousarray(W_hash, dtype=np.float32)
    bh_f = np.ascontiguousarray(b_hash, dtype=np.float32).reshape(C, 1)
    gamma_f = np.ascontiguousarray(ln_gamma, dtype=np.float32).reshape(1, M)
    beta_f = np.ascontiguousarray(ln_beta, dtype=np.float32).reshape(1, M)
    Wcls_pad = np.zeros((P, KC), dtype=np.float32)
    Wcls_pad[:C, :] = W_cls
    bcls_f = np.ascontiguousarray(b_cls, dtype=np.float32).reshape(1, KC)

    in_maps = []
    for i in range(NCORES):
        featT_i = np.ascontiguousarray(
            feat[i * B:(i + 1) * B, :].T).astype(np.float32)
        fth_i = featT_i.astype(bf)
        ftl_i = (featT_i - fth_i.astype(np.float32)).astype(bf)
        in_maps.append({
            "fth": fth_i, "ftl": ftl_i, "embT": embT, "Wk": Wk_b, "Wv": Wv_b,
            "Wqh": Wqh_b, "Wql": Wql_b, "Wh": Wh_f, "bh": bh_f,
            "gamma": gamma_f, "beta": beta_f, "Wcls": Wcls_pad, "bcls": bcls_f,
        })
    return in_maps


def kernel(feat, emb, W_k, W_v, W_q, W_hash, b_hash, ln_gamma, ln_beta,
           W_cls, b_cls):
    feat = np.asarray(feat, dtype=np.float32)
    emb = np.asarray(emb, dtype=np.float32)
    in_maps = _prep_inputs(feat, emb, np.asarray(W_k), np.asarray(W_v),
                           np.asarray(W_q), np.asarray(W_hash),
                           np.asarray(b_hash), np.asarray(ln_gamma),
                           np.asarray(ln_beta), np.asarray(W_cls),
                           np.asarray(b_cls))
    if "nc" not in _CACHED:
        _CACHED["nc"] = _build()
    res = run_bass_kernel_spmd(_CACHED["nc"], in_maps, list(range(NCORES)))
    code = np.concatenate([r["code"] for r in res.results], axis=0)
    prob = np.concatenate([r["prob"] for r in res.results], axis=0)
    fccls = np.concatenate([r["fccls"] for r in res.results], axis=0)
    return code, prob, fccls


# Exposed for test.py profiling runs
def run_profiled(feat, emb, W_k, W_v, W_q, W_hash, b_hash, ln_gamma, ln_beta,
                 W_cls, b_cls, trace=True):
    in_maps = _prep_inputs(np.asarray(feat, dtype=np.float32),
                           np.asarray(emb, dtype=np.float32),
                           np.asarray(W_k), np.asarray(W_v), np.asarray(W_q),
                           np.asarray(W_hash), np.asarray(b_hash),
                           np.asarray(ln_gamma), np.asarray(ln_beta),
                           np.asarray(W_cls), np.asarray(b_cls))
    if "nc" not in _CACHED:
        _CACHED["nc"] = _build()
    res = run_bass_kernel_spmd(_CACHED["nc"], in_maps, list(range(NCORES)),
                               trace=trace)
    code = np.concatenate([r["code"] for r in res.results], axis=0)
    prob = np.concatenate([r["prob"] for r in res.results], axis=0)
    fccls = np.concatenate([r["fccls"] for r in res.results], axis=0)
    return (code, prob, fccls), res
